# revision 5
# baseline (speedup 1.0000x reference)
"""Causal self-attention (B=4, T=2048, C=1024, H=16, HD=64) on 8 trn2 cores.

Sharding: tensor-parallel over 2 head groups x data-parallel over 4 batches
(core i: batch i%4, head group i//4, 8 heads each); host sums the two
head-group partial output projections per batch.

Design (vs the naive per-k-tile pipeline):
  - exp is batched over 2-bank PSUM "supertiles" [128, 1024] (one ACT
    instruction per 2 k-tiles) to amortize per-instruction ACT access
    overhead; ACT is the attention-phase pacer, the PE fills its gaps.
  - single-head chains with software-pipelined emission: AV lags exp by one
    duo, normalize lags its chain by one (h,j) chunk, so the PE queue never
    fronts a waiting instruction; e tiles are 6-deep so ACT can run ahead.
  - q/k projections for pair g+1 are emitted at moderately lowered
    scheduler priority (offset 3000) during pair g's attention: the list
    scheduler slots them into PE gaps instead of lumps that starve ACT.
  - the last pair's attention is j-interleaved with the output projection,
    removing the serial wo-proj tail; output partials are DMA'd in bf16.
  - xt is DMA'd in column chunks so the first q/k projection (and V group
    0) unblocks ~9us into each rep instead of after the full input stream.
  - ST/AV matmuls and exp are causally trimmed; odd heads land in a base-0
    temp + one SBUF-to-SBUF DMA (matmul partition bases must be 0/32/64).
"""

import json as _json

import numpy as np
import ml_dtypes

import concourse.bass as bass
import concourse.mybir as mybir
import concourse.bass2jax as _b2j
import concourse.bass_utils as _bu
from concourse import tile
from concourse.bass_utils import compile_bir_kernel as _orig_compile_bir_kernel

# ---------------------------------------------------------------------------
# Workaround: the neuronxcc walrus in this container rejects more than one
# sync-wait command per instruction.  Hoist extra waits onto NoOp carriers.
_BIR_MAXW = 1
import os as _os
_BIR_MAXW_COMPUTE = int(_os.environ.get("BIR_MAXW_COMPUTE", "1"))


def _split_bir_waits(bir_str):
    j = _json.loads(bir_str)
    ctr = 0
    for fn in j.get("functions", []):
        for blk in fn.get("blocks", []):
            new_insts = []
            for ins in blk["instructions"]:
                si = ins.get("sync_info")
                waits = (si or {}).get("on_wait") or []
                maxw = _BIR_MAXW if ins["opcode"] in ("Drain", "NoOp") else _BIR_MAXW_COMPUTE
                if len(waits) > maxw:
                    extra, keep = waits[:-maxw], waits[-maxw:]
                    for i in range(0, len(extra), _BIR_MAXW):
                        ctr += 1
                        new_insts.append(
                            {
                                "debug": ins.get("debug", 0),
                                "engine": ins["engine"],
                                "ins": [],
                                "name": f"{ins['name']}-sw{ctr}",
                                "opcode": "NoOp",
                                "outs": [],
                                "sync_info": {
                                    "on_update": [],
                                    "on_wait": extra[i : i + _BIR_MAXW],
                                },
                            }
                        )
                    si["on_wait"] = keep
                new_insts.append(ins)
            blk["instructions"] = new_insts
    return _json.dumps(j).encode()


def _patched_compile_bir_kernel(ant_bir_str, *args, **kwargs):
    ant_bir_str = _split_bir_waits(ant_bir_str)
    return _orig_compile_bir_kernel(ant_bir_str, *args, **kwargs)


_b2j.compile_bir_kernel = _patched_compile_bir_kernel
_bu.compile_bir_kernel = _patched_compile_bir_kernel
# ---------------------------------------------------------------------------

F32 = mybir.dt.float32
BF16 = mybir.dt.bfloat16
BF16_NP = ml_dtypes.bfloat16

N_EMBED = 1024
N_HEADS = 16
B = 4
HD = 64
CHUNK = 512  # q-chunk width (PSUM bank / max matmul N)


def build_nc(
    T=2048,
    n_heads_local=8,
    reps=1,
    phases=3,
    duo=2,
    e_bufs=6,
    causal=True,
    out_bf16=True,
    low_prio_proj=True,
    chunk_xt=True,
    trim_exp=True,
    low_prio_v=False,
    deep_w=False,
    proj_off=3000,
    st_bufs=2,
    av_bufs=2,
    pp_bufs=2,
):
    """One-core program; SPMD across 8 cores with per-core inputs."""
    C = N_EMBED
    HL = n_heads_local
    DL = HL * HD  # local head dim total (512)
    NC_C = C // 128  # 8 c-tiles
    NQK = 2 * DL // 128  # 8 d'-tiles for q|k
    NT_T = T // 128  # 16 t-subtiles for V
    NJ = T // CHUNK  # 4 q chunks
    VW = HL * (HD + 1)  # vp tile width (520): per head [v(64) | 1]
    SW = duo * CHUNK  # supertile width
    ODT = BF16 if out_bf16 else F32

    nc = bass.Bass()
    xt_d = nc.dram_tensor("xt", [C, T], BF16, kind="ExternalInput")
    wqk_d = nc.dram_tensor("wqk", [C, 2 * DL], BF16, kind="ExternalInput")
    wv_d = nc.dram_tensor("wv", [C, DL], BF16, kind="ExternalInput")
    wot_d = nc.dram_tensor("wot", [DL, C], BF16, kind="ExternalInput")
    yt_d = nc.dram_tensor("yt", [C, T], ODT, kind="ExternalOutput")

    with tile.TileContext(nc) as tc:
        with (
            tc.tile_pool(name="px", bufs=NC_C + (4 if deep_w else 0)) as px,
            tc.tile_pool(name="pwqk", bufs=2 * NC_C if deep_w else NC_C) as pwqk,
            tc.tile_pool(name="pwv", bufs=NC_C) as pwv,
            tc.tile_pool(name="pwot", bufs=DL // 128) as pwot,
            tc.tile_pool(name="pqk", bufs=NQK) as pqk,
            tc.tile_pool(name="pvp", bufs=NT_T + 4) as pvp,
            tc.tile_pool(name="pe", bufs=e_bufs) as pe,
            tc.tile_pool(name="pot", bufs=DL // 128) as pot,
            tc.tile_pool(name="pr", bufs=2) as pr,
            tc.tile_pool(name="pbc", bufs=2) as pbc,
            tc.tile_pool(name="pysb", bufs=4) as pysb,
            tc.tile_pool(name="pones", bufs=1) as pones,
            tc.tile_pool(name="psum", bufs=2, space="PSUM") as psum,
        ):
          import contextlib
          loop_ctx = tc.For_i(0, reps, 1) if reps > 1 else contextlib.nullcontext()
          with loop_ctx:
            # --- load inputs -------------------------------------------------
            # Order matters for pipeline fill: wqk + the first xt column
            # chunk unblock the first q/k projection (and V group 0) at ~9us
            # instead of waiting for the full 7MB input stream.
            xts, wqks, wvs = [], [], []
            for c in range(NC_C):
                wqk = pwqk.tile([128, 2 * DL], BF16, tag="wqk")
                nc.sync.dma_start(wqk[:], wqk_d[c * 128 : (c + 1) * 128, :])
                wqks.append(wqk)
                xt = px.tile([128, T], BF16, tag="xt")
                xts.append(xt)
            if chunk_xt:
                for c in range(NC_C):
                    nc.sync.dma_start(
                        xts[c][:, :CHUNK], xt_d[c * 128 : (c + 1) * 128, :CHUNK]
                    )
            for c in range(NC_C):
                wv = pwv.tile([128, DL], BF16, tag="wv")
                nc.sync.dma_start(wv[:], wv_d[c * 128 : (c + 1) * 128, :])
                wvs.append(wv)
            if chunk_xt:
                for jp in range(1, NJ):
                    for c in range(NC_C):
                        nc.sync.dma_start(
                            xts[c][:, jp * CHUNK : (jp + 1) * CHUNK],
                            xt_d[c * 128 : (c + 1) * 128, jp * CHUNK : (jp + 1) * CHUNK],
                        )
            else:
                for c in range(NC_C):
                    nc.sync.dma_start(xts[c][:], xt_d[c * 128 : (c + 1) * 128, :])
            ones = pones.tile([128, 128], BF16, tag="ones")
            nc.gpsimd.memset(ones[:], 1.0)

            # --- V projection ([v(64) | 1] per head) ------------------------
            import contextlib as _ctxlib

            vps = []
            for ts in range(NT_T):
                vprio = (
                    tc.high_priority(offset=-1000000)
                    if (low_prio_v and ts >= 4)
                    else _ctxlib.nullcontext()
                )
                with vprio:
                    vp = pvp.tile([128, VW], BF16, tag="vp")
                    ps = psum.tile([128, CHUNK], F32, tag="pp", bufs=pp_bufs, name=f"vps{ts}")
                    for c in range(NC_C):
                        nc.tensor.matmul(
                            ps[:],
                            xts[c][:, ts * 128 : (ts + 1) * 128],
                            wvs[c][:],
                            start=(c == 0),
                            stop=(c == NC_C - 1),
                        )
                    ps3 = ps[:].rearrange("p (h c) -> p h c", c=HD)
                    vp3 = vp[:].rearrange("p (h c) -> p h c", c=HD + 1)
                    nc.vector.tensor_copy(vp3[:, :, 0:HD], ps3[:, :, :])
                    nc.gpsimd.memset(vp3[:, :, HD : HD + 1], 1.0)
                    vps.append(vp)

            # --- q/k projections (transposed layout) -------------------------
            def project_qk(dq):
                qk = pqk.tile([128, T], BF16, tag="qk", name=f"qk{dq}")
                for jp in range(NJ):
                    ps = psum.tile(
                        [128, CHUNK], F32, tag="pp", bufs=pp_bufs, name=f"qkps{dq}_{jp}"
                    )
                    for c in range(NC_C):
                        nc.tensor.matmul(
                            ps[:],
                            wqks[c][:, dq * 128 : (dq + 1) * 128],
                            xts[c][:, jp * CHUNK : (jp + 1) * CHUNK],
                            start=(c == 0),
                            stop=(c == NC_C - 1),
                        )
                    nc.vector.tensor_copy(
                        qk[:, jp * CHUNK : (jp + 1) * CHUNK], ps[:]
                    )
                return qk

            if phases == 1:
                assert not out_bf16
                for dq in range(NQK):
                    qkx = project_qk(dq)
                    nc.sync.dma_start(
                        yt_d[dq * 128 : (dq + 1) * 128, : T // 2].bitcast(BF16),
                        qkx[:],
                    )
                return nc

            ots = [
                pot.tile([128, T], BF16, tag="ot", name=f"ot{i}")
                for i in range(DL // 128)
            ]
            kpb = CHUNK // 128  # k-tiles per q chunk (4)

            qs, ks = {}, {}
            tmps = {}

            def emit_proj(g):
                qs[g] = project_qk(g)
                ks[g] = project_qk(NQK // 2 + g)
                tmps[g] = pot.tile([64, T], BF16, tag="ottmp", name=f"ottmp{g}")

            def vp_slice(vp, h):
                return vp[:, h * (HD + 1) : (h + 1) * (HD + 1)]

            def emit_core(h, j):
                """ST/exp/mask/AV for chunk j of head h; returns av tile."""
                g, par = h // 2, h % 2
                po = par * 64
                qk_q, qk_k = qs[g], ks[g]
                av = psum.tile([128, CHUNK], F32, tag="av", bufs=av_bufs, name=f"av{h}_{j}")
                avs = av[0 : HD + 1]
                n_kt = kpb * (j + 1) if causal else T // 128
                pend = []  # lag AV one duo behind exp
                for d0kt in range(0, n_kt, duo):
                    nd = min(duo, n_kt - d0kt)
                    st = psum.tile([128, SW], F32, tag="st", bufs=st_bufs, name=f"st{h}_{j}_{d0kt}")
                    for m in range(nd):
                        kt = d0kt + m
                        diag = causal and kt >= kpb * j
                        d0 = (kt - kpb * j) * 128 if diag else 0
                        nc.tensor.matmul(
                            st[:, m * CHUNK + d0 : (m + 1) * CHUNK],
                            qk_k[po : po + 64, kt * 128 : (kt + 1) * 128],
                            qk_q[po : po + 64, j * CHUNK + d0 : (j + 1) * CHUNK],
                            start=True,
                            stop=True,
                        )
                    e = pe.tile([128, SW], BF16, tag="e", name=f"e{h}_{j}_{d0kt}")
                    # skip the leading below-diagonal columns of subtile 0
                    off0 = (
                        (d0kt - kpb * j) * 128
                        if (trim_exp and causal and d0kt >= kpb * j)
                        else 0
                    )
                    nc.scalar.activation(
                        e[:, off0 : nd * CHUNK],
                        st[:, off0 : nd * CHUNK],
                        mybir.ActivationFunctionType.Exp,
                        scale=float(HD) ** -0.5,
                    )
                    for m in range(nd):
                        kt = d0kt + m
                        diag = causal and kt >= kpb * j
                        if diag:
                            d0 = (kt - kpb * j) * 128
                            nc.gpsimd.affine_select(
                                out=e[:, m * CHUNK + d0 : m * CHUNK + d0 + 128],
                                in_=e[:, m * CHUNK + d0 : m * CHUNK + d0 + 128],
                                compare_op=mybir.AluOpType.is_ge,
                                fill=0.0,
                                base=0,
                                pattern=[[1, 128]],
                                channel_multiplier=-1,
                            )
                    if pend:
                        pend.pop(0)()
                    def mk_av(e=e, d0kt=d0kt, nd=nd):
                        for m in range(nd):
                            kt = d0kt + m
                            diag = causal and kt >= kpb * j
                            d0 = (kt - kpb * j) * 128 if diag else 0
                            nc.tensor.matmul(
                                avs[:, d0:CHUNK],
                                vp_slice(vps[kt], h),
                                e[:, m * CHUNK + d0 : (m + 1) * CHUNK],
                                start=(kt == 0),
                                stop=(kt == n_kt - 1),
                            )
                    pend.append(mk_av)
                for fn in pend:
                    fn()
                return av

            def emit_norm(h, j, av):
                g, par = h // 2, h % 2
                r = pr.tile([128, CHUNK], BF16, tag="r", name=f"r{h}_{j}")
                with nc.allow_low_precision("bf16 softmax denom (~4e-3 ok)"):
                    nc.vector.reciprocal(r[64:65, :], av[64:65, :])
                bc = psum.tile([128, CHUNK], F32, tag="pp", bufs=pp_bufs, name=f"bc{h}_{j}")
                nc.tensor.matmul(
                    bc[0:64, :], ones[64:65, 0:64], r[64:65, :],
                    start=True, stop=True,
                )
                bcs = pbc.tile([128, CHUNK], F32, tag="bc", name=f"bcs{h}_{j}")
                nc.vector.tensor_copy(bcs[0:64, :], bc[0:64, :])
                mul_out = (
                    ots[g][0:64, j * CHUNK : (j + 1) * CHUNK]
                    if par == 0
                    else tmps[g][:, j * CHUNK : (j + 1) * CHUNK]
                )
                nc.vector.tensor_mul(mul_out, av[0:64, :], bcs[0:64, :])
                if par == 1 and j == NJ - 1 and (phases == 2 or g < HL // 2 - 1):
                    nc.sync.dma_start(ots[g][64:128, :], tmps[g][:])

            wots = []

            def emit_wo_chunk(j):
                for e_t in range(C // 128):
                    yp = psum.tile([128, CHUNK], F32, tag="pp", bufs=pp_bufs, name=f"yp{e_t}_{j}")
                    for d in range(DL // 128):
                        nc.tensor.matmul(
                            yp[:],
                            wots[d][:, e_t * 128 : (e_t + 1) * 128],
                            ots[d][:, j * CHUNK : (j + 1) * CHUNK],
                            start=(d == 0),
                            stop=(d == DL // 128 - 1),
                        )
                    ysb = pysb.tile([128, CHUNK], ODT, tag="ysb")
                    nc.vector.tensor_copy(ysb[:], yp[:])
                    nc.sync.dma_start(
                        yt_d[e_t * 128 : (e_t + 1) * 128, j * CHUNK : (j + 1) * CHUNK],
                        ysb[:],
                    )

            # --- attention: chains with lag-1 normalize, proj interleaved ---
            # Interleaved projections are emitted at low scheduler priority so
            # the PE treats them as gap fillers while ACT paces the exp chain.
            import contextlib

            def proj_prio():
                return (
                    tc.high_priority(offset=-proj_off)
                    if low_prio_proj
                    else contextlib.nullcontext()
                )

            emit_proj(0)
            pending_norm = None
            G = HL // 2
            n_plain = G if phases == 2 else G - 1
            for g in range(n_plain):
                for par in range(2):
                    h = 2 * g + par
                    for j in range(NJ):
                        av = emit_core(h, j)
                        if pending_norm is not None:
                            emit_norm(*pending_norm)
                        pending_norm = (h, j, av)
                    if par == 0 and g + 1 < G:
                        with proj_prio():
                            emit_proj(g + 1)

            if phases == 2:
                assert not out_bf16
                if pending_norm is not None:
                    emit_norm(*pending_norm)
                for i, ot in enumerate(ots):
                    nc.sync.dma_start(
                        yt_d[i * 128 : (i + 1) * 128, : T // 2].bitcast(BF16),
                        ot[:],
                    )
                return nc

            # --- last pair: attention j-interleaved with output projection --

            for d in range(DL // 128):
                wot = pwot.tile([128, C], BF16, tag="wot")
                nc.sync.dma_start(wot[:], wot_d[d * 128 : (d + 1) * 128, :])
                wots.append(wot)
            gB = G - 1
            hA, hB = 2 * gB, 2 * gB + 1
            for j in range(NJ):
                avA = emit_core(hA, j)
                if pending_norm is not None:
                    emit_norm(*pending_norm)
                    pending_norm = None
                avB = emit_core(hB, j)
                emit_norm(hA, j, avA)
                if j > 0:
                    emit_wo_chunk(j - 1)
                emit_norm(hB, j, avB)
                nc.sync.dma_start(
                    ots[gB][64:128, j * CHUNK : (j + 1) * CHUNK],
                    tmps[gB][:, j * CHUNK : (j + 1) * CHUNK],
                )
            emit_wo_chunk(NJ - 1)
    return nc


_CACHE = {}


def _get_nc(T, n_heads_local):
    key = (T, n_heads_local)
    if key not in _CACHE:
        _CACHE[key] = build_nc(T, n_heads_local)
    return _CACHE[key]


def make_in_maps(x, wq, wk, wv, wo):
    x = np.asarray(x, dtype=np.float32)
    wq = np.asarray(wq, dtype=np.float32)
    wk = np.asarray(wk, dtype=np.float32)
    wv = np.asarray(wv, dtype=np.float32)
    wo = np.asarray(wo, dtype=np.float32)
    HL = N_HEADS // 2
    DL = HL * HD
    in_maps = []
    for core in range(8):
        bi = core % 4
        g = core // 4
        gs = slice(g * DL, (g + 1) * DL)
        xt = np.ascontiguousarray(x[bi].T).astype(BF16_NP)
        wqk = np.concatenate([wq[gs].T, wk[gs].T], axis=1).astype(BF16_NP)
        wvt = np.ascontiguousarray(wv[gs].T).astype(BF16_NP)
        wot = np.ascontiguousarray(wo[:, gs].T).astype(BF16_NP)
        in_maps.append(
            {
                "xt": np.ascontiguousarray(xt),
                "wqk": np.ascontiguousarray(wqk),
                "wv": wvt,
                "wot": wot,
            }
        )
    return in_maps


def run(x, wq, wk, wv, wo, trace=False):
    from concourse.bass_utils import run_bass_kernel_spmd

    b, T, C = np.asarray(x).shape
    HL = N_HEADS // 2

    nc = _get_nc(T, HL)
    in_maps = make_in_maps(x, wq, wk, wv, wo)
    try:
        res = run_bass_kernel_spmd(nc, in_maps, list(range(8)), trace=trace)
    except ModuleNotFoundError:
        res = run_bass_kernel_spmd(nc, in_maps, list(range(8)), trace=False)
    y = np.empty((b, T, C), dtype=np.float32)
    for bi in range(b):
        yt = np.asarray(res.results[bi]["yt"], dtype=np.float32) + np.asarray(
            res.results[bi + 4]["yt"], dtype=np.float32
        )
        y[bi] = yt.T
    return y, res


def kernel(x, wq, wk, wv, wo):
    y, _ = run(x, wq, wk, wv, wo, trace=False)
    return y
